# revision 21
# baseline (speedup 1.0000x reference)
"""Trainium2 Bass kernel for GNN message passing:
    out[i] = sum_{e: dst[e]==i} x[src[e]]     (x: [N, 64] f32, edge_index: [2, E] int)

Strategy (graph-partitioned node sharding, 8 cores):
  * Host sorts edges by destination and shards the destination-node space
    across the 8 cores (N/8 nodes per core, x replicated). Each core's
    128-node destination tiles are permuted so heavy tiles align across
    cores (minimizes union padding; host un-permutes rows at the end).
  * x is repacked as [N, 128] bf16 rows: [bf16(x) | bf16(x - bf16(x))]
    (hi|lo split): one 256 B-row gather fetches both halves, one bf16
    matmul per chunk processes both, and they are summed at evacuation —
    ~1e-5 relative accuracy at bf16 PE speed.
  * Edges are grouped per (supertile of 8 dst tiles, source block of 25000
    rows — int16-safe for dma_gather) into contiguous runs, padded only to
    the 128-edge chunk size. Chunks may straddle destination-tile
    boundaries; every (chunk, tile) pair present in ANY core gets a matmul
    slot, and per-core local-dst streams mask foreign edges with -1.
  * Gather descriptor emission (SWDGE, one Q7 core pair per queue) is the
    bottleneck; calls round-robin across 4 SWDGE queues so all four core
    pairs emit concurrently.  Every call passes the SAME num_idxs register
    (hoisted, written once) and a full-width idx stream padded with
    trailing -1s (the ucode trims them), so no per-call register MOVE
    creates a WAR hazard that would serialize dispatch.
  * Per core, per chunk: dma_gather (GPSIMD) fetches packed rows; VectorE
    builds [128,128] bf16 one-hots (fused 4 slots per tensor_tensor
    is_equal against a replicated iota); TensorE accumulates
    psum[tile] += onehot.T @ msgs (one PSUM bank per live tile);
    ScalarE+VectorE merge hi+lo into SBUF staging at supertile end.
  * Each core stores its padded [N/8, 64] f32 slice with one DMA; the host
    un-permutes tile rows and concatenates. No collectives.
"""

import numpy as np
import ml_dtypes

import concourse.bacc as bacc
import concourse.bass as bass
import concourse.mybir as mybir
import concourse.tile as tile
from concourse.bass_utils import run_bass_kernel_spmd

P = 128
F32 = mybir.dt.float32
BF16 = mybir.dt.bfloat16
I16 = mybir.dt.int16
I32 = mybir.dt.int32
BF = ml_dtypes.bfloat16

# Full-problem constants (hardcoded per harness contract).
N_NODES = 100000
DIM = 64
N_CORES = 8
SRC_BLOCK = 25000        # int16-safe source block
CHUNKS_PER_CALL = 12     # max chunks per dma_gather call; split packets
SUPERTILE = 8            # dst tiles per supertile (<= 8 PSUM banks live)
N_QUEUES = 4             # SWDGE queues (each runs on its own Q7 core pair)
SINGLE_PACKET = False    # single_packet caps at 64 ring descriptors
# core pair 0 is measurably slower per call (extra per-call duties), so
# it gets 1/7 of calls instead of 1/4
QUEUE_PATTERN = [0, 1, 2, 3]
SCALAR_PCT = 20          # %% of one-hot slots built on ScalarE (Square+Relu)


def _prep(edge_index, n_nodes, n_cores, block, w, stile=SUPERTILE):
    npc = n_nodes // n_cores
    tiles = -(-npc // P)
    nblocks = -(-n_nodes // block)
    n_super = -(-tiles // stile)

    dst = np.asarray(edge_index[0]).astype(np.int64)
    src = np.asarray(edge_index[1]).astype(np.int64)

    k_of = dst // npc
    t_of = (dst - k_of * npc) // P
    b_of = src // block
    seg = (k_of * tiles + t_of) * nblocks + b_of
    order = np.argsort(seg, kind="stable")
    dst_s = dst[order]
    src_s = src[order]
    seg_s = seg[order]

    counts0 = np.bincount(
        seg_s, minlength=n_cores * tiles * nblocks
    ).reshape(n_cores, tiles, nblocks)
    # global start offset of each (core, true tile, block) bucket
    all_starts = np.concatenate(
        [[0], np.cumsum(counts0.ravel())]
    )

    # tile -> slot permutation per core (align heavy tiles across cores)
    perm = np.argsort(-counts0.sum(axis=2), axis=1, kind="stable")  # [cores, tiles]
    counts = np.take_along_axis(counts0, perm[:, :, None], axis=1)  # [cores,slot,b]

    # ---- runs: (supertile, block) -> concatenated slot buckets, pad to 128
    chunk_block = []
    chunk_super = []
    run_meta = []  # (s, b, chunk0, nch, ends_k [cores, nts], ts)
    for s in range(n_super):
        ts = list(range(s * stile, min((s + 1) * stile, tiles)))
        for b in range(nblocks):
            c_kt = counts[:, ts, b]                      # [cores, nts]
            run_max = int(c_kt.sum(axis=1).max())
            if run_max == 0:
                continue
            nch = -(-run_max // P)
            chunk0 = len(chunk_block)
            chunk_block += [b] * nch
            chunk_super += [s] * nch
            run_meta.append((s, b, chunk0, nch, np.cumsum(c_kt, axis=1), ts))
    ch = len(chunk_block)
    chunk_block = np.array(chunk_block)
    chunk_super = np.array(chunk_super)

    # ---- matmul slots: union over cores of tiles present in each chunk
    mm_chunk = []
    mm_tile = []
    for s, b, chunk0, nch, ends_k, ts in run_meta:
        starts_k = ends_k - counts[:, ts, b]
        for ci_local in range(nch):
            a0, a1 = ci_local * P, (ci_local + 1) * P
            present = ((starts_k < a1) & (ends_k > a0)).any(axis=0)
            for j in np.nonzero(present)[0]:
                mm_chunk.append(chunk0 + ci_local)
                mm_tile.append(ts[j])
    nslots = len(mm_chunk)
    mm_chunk = np.array(mm_chunk)
    mm_tile = np.array(mm_tile)

    mm_first = np.zeros(nslots, dtype=bool)
    mm_last = np.zeros(nslots, dtype=bool)
    seen = set()
    for i in range(nslots):
        t = int(mm_tile[i])
        if t not in seen:
            seen.add(t)
            mm_first[i] = True
    seen = set()
    for i in range(nslots - 1, -1, -1):
        t = int(mm_tile[i])
        if t not in seen:
            seen.add(t)
            mm_last[i] = True
    tile_has = np.zeros(tiles, dtype=bool)
    if nslots:
        tile_has[np.unique(mm_tile)] = True

    # ---- calls: one call per run; two padded sizes (w_lo/w_hi) minimize
    # zero-pad emission while keeping num_idxs a shared constant per size
    sizes = sorted({nch for _, _, _, nch, _, _ in run_meta})
    w_hi = max(sizes)
    best = (None, None)
    for wl in sizes:
        pad = sum(
            (wl - nch) if nch <= wl else (w_hi - nch)
            for _, _, _, nch, _, _ in run_meta
        )
        if best[0] is None or pad < best[0]:
            best = (pad, wl)
    w_lo = best[1]
    calls = []  # (block, c0, csize, slot0, nslots_call, wcall, off)
    off = 0
    for s, b, chunk0, nch, ends_k, ts in run_meta:
        s0 = int(np.searchsorted(mm_chunk, chunk0))
        s1 = int(np.searchsorted(mm_chunk, chunk0 + nch))
        wcall = w_lo if nch <= w_lo else w_hi
        calls.append((b, chunk0, nch, s0, s1 - s0, wcall, off))
        off += wcall
    total_w = off
    max_slots_call = max(c[4] for c in calls)

    # ---- per-core streams
    idx_flat = np.zeros((n_cores, ch * P), np.int16)
    ldst_slots = np.full((n_cores, nslots, P), -1.0, BF)
    for k in range(n_cores):
        for s, b, chunk0, nch, ends_k, ts in run_meta:
            pieces_src = []
            pieces_ldst = []
            pieces_slot = []
            for j, t in enumerate(ts):
                cnt = int(counts[k, t, b])
                if cnt == 0:
                    continue
                tt = int(perm[k, t])
                g0 = int(all_starts[(k * tiles + tt) * nblocks + b])
                pieces_src.append(src_s[g0 : g0 + cnt] - b * block)
                pieces_ldst.append(dst_s[g0 : g0 + cnt] - (k * npc + tt * P))
                pieces_slot.append(np.full(cnt, t, np.int64))
            if not pieces_src:
                continue
            esrc = np.concatenate(pieces_src).astype(np.int16)
            eldst = np.concatenate(pieces_ldst)
            eslot = np.concatenate(pieces_slot)
            n_e = esrc.shape[0]
            base = chunk0 * P
            idx_flat[k, base : base + n_e] = esrc
            s0 = int(np.searchsorted(mm_chunk, chunk0))
            s1 = int(np.searchsorted(mm_chunk, chunk0 + nch))
            for i in range(s0, s1):
                ci_local = int(mm_chunk[i]) - chunk0
                t = int(mm_tile[i])
                a0 = ci_local * P
                a1 = min(a0 + P, n_e)
                if a0 >= n_e:
                    continue
                m = eslot[a0:a1] == t
                if not m.any():
                    continue
                col = ldst_slots[k, i]
                col[: a1 - a0][m] = eldst[a0:a1][m].astype(BF)

    # wrap indices into 16 partitions, replicate across the 8 core groups
    idx_wrapped = np.tile(
        idx_flat.reshape(n_cores, ch * 8, 16).transpose(0, 2, 1), (1, 8, 1)
    )  # [cores, 128, ch*8]

    # per-call full-width idx stream: call j occupies cols [j*w*8,(j+1)*w*8);
    # tail cols beyond csize*8 are 0 (a valid row — gathered then discarded;
    # trailing -1 trim corrupts ring bookkeeping when queues are reused), so
    # every call passes the same num_idxs = w*128 and shares one register.
    ncalls = len(calls)
    idx_calls = np.zeros((n_cores, P, total_w * 8), np.int16)
    for b, c0, csize, s0, nsc, wcall, off in calls:
        idx_calls[:, :, off * 8 : off * 8 + csize * 8] = idx_wrapped[
            :, :, c0 * 8 : (c0 + csize) * 8
        ]
    idx_all = np.ascontiguousarray(idx_calls)

    # ldst packed at a fixed per-call stride (msc slots) so meta DMAs can
    # fetch G calls' worth in one transfer
    ldst_t_slots = ldst_slots.transpose(0, 2, 1)  # [cores, P, nslots]
    ldst_pack = np.full((n_cores, P, ncalls * max_slots_call), -1.0, BF)
    for j, (b, c0, csize, s0, nsc, wcall, off) in enumerate(calls):
        if nsc:
            ldst_pack[:, :, j * max_slots_call : j * max_slots_call + nsc] = (
                ldst_t_slots[:, :, s0 : s0 + nsc]
            )
            # slots offloaded to ScalarE (last ~20% per call) store -l so the
            # activation bias computes (iota - l) directly
            v_cnt = nsc - nsc * SCALAR_PCT // 100
            if v_cnt < nsc:
                cols = slice(j * max_slots_call + v_cnt, j * max_slots_call + nsc)
                ldst_pack[:, :, cols] = -ldst_pack[:, :, cols]
    ldst_all = np.ascontiguousarray(ldst_pack)

    return dict(
        npc=npc,
        tiles=tiles,
        nblocks=nblocks,
        n_super=n_super,
        stile=stile,
        ch=ch,
        nslots=nslots,
        calls=calls,
        max_slots_call=max_slots_call,
        chunk_super=chunk_super,
        mm_chunk=mm_chunk,
        mm_tile=mm_tile,
        mm_first=mm_first,
        mm_last=mm_last,
        tile_has_chunks=tile_has,
        idx=idx_all,
        w_lo=w_lo,
        w_hi=w_hi,
        total_w=total_w,
        ldst=ldst_all,
        perm=perm,
    )


def _pack_x(x):
    """[N, D] f32 -> [N, 2D] bf16 rows: [hi | lo]."""
    x = np.asarray(x, np.float32)
    hi = x.astype(BF)
    lo = (x - hi.astype(np.float32)).astype(BF)
    return np.ascontiguousarray(np.concatenate([hi, lo], axis=1))


def _build(n_nodes, dim, block, w, sched):
    w_lo = sched["w_lo"]
    w_hi = sched["w_hi"]
    total_w = sched["total_w"]
    tiles = sched["tiles"]
    stile = sched["stile"]
    n_super = sched["n_super"]
    nslots = sched["nslots"]
    calls = sched["calls"]
    msc = sched["max_slots_call"]
    chunk_super = sched["chunk_super"]
    mm_chunk = sched["mm_chunk"]
    mm_tile = sched["mm_tile"]
    mm_first = sched["mm_first"]
    mm_last = sched["mm_last"]
    tile_has = sched["tile_has_chunks"]
    ncalls = len(calls)
    out_pad = tiles * P
    elem = 2 * dim  # packed bf16 row length

    nc = bacc.Bacc(
        "TRN2", target_bir_lowering=False, debug=False, num_swdge_queues=N_QUEUES
    )
    x_t = nc.dram_tensor("xpack", [n_nodes, elem], BF16, kind="ExternalInput")
    idx_t = nc.dram_tensor("idx", [P, total_w * 8], I16, kind="ExternalInput")
    ldst_t = nc.dram_tensor("ldst", [P, ncalls * msc], BF16, kind="ExternalInput")
    out_t = nc.dram_tensor("out", [out_pad, dim], F32, kind="ExternalOutput")
    G = 4  # calls per meta DMA group

    with tile.TileContext(nc) as tc:
        with (
            tc.tile_pool(name="const", bufs=1) as const_pool,
            tc.tile_pool(name="meta", bufs=8) as meta_pool,
            tc.tile_pool(name="gather", bufs=8) as gather_pool,
            tc.tile_pool(name="oh", bufs=14) as oh_pool,
            tc.tile_pool(name="sq", bufs=16) as sq_pool,
            tc.tile_pool(name="stage", bufs=1) as stage_pool,
            tc.tile_pool(name="psum", bufs=8, space="PSUM") as psum_pool,
        ):
            iota_i = const_pool.tile([P, 8 * P], I32)
            nc.gpsimd.iota(
                iota_i[:], pattern=[[0, 8], [1, P]], base=0, channel_multiplier=0
            )
            iota_b = const_pool.tile([P, 8 * P], BF16)
            nc.vector.tensor_copy(iota_b[:], iota_i[:])

            stage = stage_pool.tile([P, tiles * dim], F32)
            if not tile_has.all():
                nc.vector.memset(stage[:], 0.0)
            out_view = out_t[:, :].rearrange("(t p) d -> p t d", p=P)

            # shared num_idxs registers: written once, read by every
            # gather -> no per-call MOVE / WAR hazard serializing dispatch
            reg_lo = nc.gpsimd.to_reg(w_lo * P)
            reg_hi = nc.gpsimd.to_reg(w_hi * P)

            call_idx = 0
            psums = {}
            pending_evac = None

            def flush_evac():
                nonlocal pending_evac
                if pending_evac is None:
                    return
                ts_p = pending_evac
                pending_evac = None
                for t in ts_p:
                    if not tile_has[t]:
                        continue
                    ps = psums.pop(t)
                    nc.scalar.copy(
                        stage[:, t * dim : (t + 1) * dim], ps[:, :dim]
                    )
                    nc.vector.tensor_tensor(
                        out=stage[:, t * dim : (t + 1) * dim],
                        in0=stage[:, t * dim : (t + 1) * dim],
                        in1=ps[:, dim:],
                        op=mybir.AluOpType.add,
                    )
                t0, t1 = ts_p[0], ts_p[-1] + 1
                nc.sync.dma_start(
                    out_view[:, t0:t1, :],
                    stage[:, t0 * dim : t1 * dim].rearrange(
                        "p (t d) -> p t d", d=dim
                    ),
                )

            for s in range(n_super):
                ts = list(range(s * stile, min((s + 1) * stile, tiles)))
                cis = 0
                while call_idx < len(calls):
                    b, c0, csize, s0, nsc, wcall, off = calls[call_idx]
                    if int(chunk_super[c0]) != s:
                        break
                    queue = QUEUE_PATTERN[call_idx % len(QUEUE_PATTERN)]
                    j = call_idx
                    call_idx += 1
                    cis += 1
                    if j % G == 0:
                        ng = min(G, ncalls - j)
                        goff = off
                        gw = sum(c[5] for c in calls[j : j + ng])
                        idx_gtile = meta_pool.tile(
                            [P, G * w_hi * 8], I16, tag="idx"
                        )
                        nc.sync.dma_start(
                            idx_gtile[:, : gw * 8],
                            idx_t[:, goff * 8 : (goff + gw) * 8],
                        )
                        ldst_gtile = meta_pool.tile([P, G * msc], BF16, tag="ldst")
                        nc.sync.dma_start(
                            ldst_gtile[:, : ng * msc],
                            ldst_t[:, j * msc : (j + ng) * msc],
                        )
                    jg = j % G
                    coff = off - goff
                    msgs = gather_pool.tile([P, w_hi, elem], BF16)
                    nc.gpsimd.dma_gather(
                        out_ap=msgs[:, :wcall, :],
                        in_ap=x_t[b * block : min((b + 1) * block, n_nodes), :],
                        idxs_ap=idx_gtile[:, coff * 8 : (coff + wcall) * 8],
                        num_idxs=wcall * P,
                        num_idxs_reg=reg_lo if wcall == w_lo else reg_hi,
                        elem_size=elem,
                        single_packet=SINGLE_PACKET,
                        queue_num=queue,
                    )
                    if cis == 2:
                        flush_evac()
                    v_cnt = nsc - nsc * SCALAR_PCT // 100

                    def do_mm(si, lhsT_ap, cin):
                        t = int(mm_tile[si])
                        if mm_first[si]:
                            psums[t] = psum_pool.tile(
                                [P, elem], F32, tag="ps", name=f"ps{t}"
                            )
                        nc.tensor.matmul(
                            psums[t][:, :],
                            lhsT=lhsT_ap,
                            rhs=msgs[:, cin, :],
                            start=bool(mm_first[si]),
                            stop=bool(mm_last[si]),
                        )

                    for j0 in range(0, v_cnt, 8):
                        g = min(8, v_cnt - j0)
                        onehot = oh_pool.tile([P, 8 * P], BF16, name="oh", tag="oh")
                        lt = ldst_gtile[:, jg * msc + j0 : jg * msc + j0 + g]
                        lt_b = bass.AP(lt.tensor, lt.offset, lt.ap + [[0, P]])
                        nc.vector.tensor_tensor(
                            out=onehot[:, : g * P].rearrange(
                                "p (g q) -> p g q", q=P
                            ),
                            in0=iota_b[:, : g * P].rearrange(
                                "p (g q) -> p g q", q=P
                            ),
                            in1=lt_b,
                            op=mybir.AluOpType.is_equal,
                        )
                        for jj in range(g):
                            si = s0 + j0 + jj
                            cin = int(mm_chunk[si]) - c0
                            do_mm(si, onehot[:, jj * P : (jj + 1) * P], cin)
                    for j0 in range(v_cnt, nsc):
                        si = s0 + j0
                        cin = int(mm_chunk[si]) - c0
                        lt_col = ldst_gtile[:, jg * msc + j0 : jg * msc + j0 + 1]
                        sq = sq_pool.tile([P, P], BF16, tag="sq")
                        nc.scalar.activation(
                            sq[:],
                            iota_b[:, :P],
                            mybir.ActivationFunctionType.Square,
                            bias=lt_col,
                        )
                        oh_s = sq_pool.tile([P, P], BF16, tag="ohs")
                        nc.scalar.activation(
                            oh_s[:],
                            sq[:],
                            mybir.ActivationFunctionType.Relu,
                            bias=1.0,
                            scale=-1.0,
                        )
                        do_mm(si, oh_s[:], cin)
                # defer this supertile's evacuation into the next
                # supertile's call stream (flush_evac)
                if pending_evac is not None:
                    flush_evac()
                pending_evac = ts
            flush_evac()

    nc.compile()
    return nc


def _run(x, edge_index, n_nodes, dim, n_cores, block, w, **run_kwargs):
    sched = _prep(edge_index, n_nodes, n_cores, block, w)
    xp = _pack_x(x)
    nc = _build(n_nodes, dim, block, sched["w_hi"], sched)
    in_maps = [
        {"xpack": xp, "idx": sched["idx"][k], "ldst": sched["ldst"][k]}
        for k in range(n_cores)
    ]
    res = run_bass_kernel_spmd(
        nc, in_maps, core_ids=list(range(n_cores)), **run_kwargs
    )
    npc = sched["npc"]
    tiles = sched["tiles"]
    perm = sched["perm"]
    parts = []
    for k in range(n_cores):
        r = res.results[k]["out"].reshape(tiles, P, -1)
        inv = np.empty(tiles, np.int64)
        inv[perm[k]] = np.arange(tiles)
        parts.append(r[inv].reshape(tiles * P, -1)[:npc])
    out = np.concatenate(parts, axis=0)
    return out, res


def kernel(x, edge_index):
    out, _ = _run(
        x, edge_index, N_NODES, DIM, N_CORES, SRC_BLOCK, CHUNKS_PER_CALL
    )
    return out


# revision 22
# speedup vs baseline: 1.0235x; 1.0235x over previous
"""Trainium2 Bass kernel for GNN message passing:
    out[i] = sum_{e: dst[e]==i} x[src[e]]     (x: [N, 64] f32, edge_index: [2, E] int)

Strategy (graph-partitioned node sharding, 8 cores):
  * Host sorts edges by destination and shards the destination-node space
    across the 8 cores (N/8 nodes per core, x replicated). Each core's
    128-node destination tiles are permuted so heavy tiles align across
    cores (minimizes union padding; host un-permutes rows at the end).
  * x is repacked as [N, 128] bf16 rows: [bf16(x) | bf16(x - bf16(x))]
    (hi|lo split): one 256 B-row gather fetches both halves, one bf16
    matmul per chunk processes both, and they are summed at evacuation —
    ~1e-5 relative accuracy at bf16 PE speed.
  * Edges are grouped per (supertile of 8 dst tiles, source block of 25000
    rows — int16-safe for dma_gather) into contiguous runs, padded only to
    the 128-edge chunk size. Chunks may straddle destination-tile
    boundaries; every (chunk, tile) pair present in ANY core gets a matmul
    slot, and per-core local-dst streams mask foreign edges with -1.
  * Gather descriptor emission (SWDGE, one Q7 core pair per queue) is the
    bottleneck; calls round-robin across 4 SWDGE queues so all four core
    pairs emit concurrently.  Every call passes the SAME num_idxs register
    (hoisted, written once) and a full-width idx stream padded with
    trailing -1s (the ucode trims them), so no per-call register MOVE
    creates a WAR hazard that would serialize dispatch.
  * Per core, per chunk: dma_gather (GPSIMD) fetches packed rows; VectorE
    builds [128,128] bf16 one-hots (fused 4 slots per tensor_tensor
    is_equal against a replicated iota); TensorE accumulates
    psum[tile] += onehot.T @ msgs (one PSUM bank per live tile);
    ScalarE+VectorE merge hi+lo into SBUF staging at supertile end.
  * Each core stores its padded [N/8, 64] f32 slice with one DMA; the host
    un-permutes tile rows and concatenates. No collectives.
"""

import numpy as np
import ml_dtypes

import concourse.bacc as bacc
import concourse.bass as bass
import concourse.mybir as mybir
import concourse.tile as tile
from concourse.bass_utils import run_bass_kernel_spmd

P = 128
F32 = mybir.dt.float32
BF16 = mybir.dt.bfloat16
I16 = mybir.dt.int16
I32 = mybir.dt.int32
BF = ml_dtypes.bfloat16

# Full-problem constants (hardcoded per harness contract).
N_NODES = 100000
DIM = 64
N_CORES = 8
SRC_BLOCK = 25000        # int16-safe source block
CHUNKS_PER_CALL = 12     # max chunks per dma_gather call; split packets
SUPERTILE = 8            # dst tiles per supertile (<= 8 PSUM banks live)
N_QUEUES = 4             # SWDGE queues (each runs on its own Q7 core pair)
SINGLE_PACKET = False    # single_packet caps at 64 ring descriptors
# core pair 0 is measurably slower per call (extra per-call duties), so
# it gets 1/7 of calls instead of 1/4
QUEUE_PATTERN = [0, 1, 2, 3]
SCALAR_PCT = 20          # %% of one-hot slots built on ScalarE (Square+Relu)


def _prep(edge_index, n_nodes, n_cores, block, w, stile=SUPERTILE):
    npc = n_nodes // n_cores
    tiles = -(-npc // P)
    nblocks = -(-n_nodes // block)
    n_super = -(-tiles // stile)

    dst = np.asarray(edge_index[0]).astype(np.int64)
    src = np.asarray(edge_index[1]).astype(np.int64)

    k_of = dst // npc
    t_of = (dst - k_of * npc) // P
    b_of = src // block
    seg = (k_of * tiles + t_of) * nblocks + b_of
    order = np.argsort(seg, kind="stable")
    dst_s = dst[order]
    src_s = src[order]
    seg_s = seg[order]

    counts0 = np.bincount(
        seg_s, minlength=n_cores * tiles * nblocks
    ).reshape(n_cores, tiles, nblocks)
    # global start offset of each (core, true tile, block) bucket
    all_starts = np.concatenate(
        [[0], np.cumsum(counts0.ravel())]
    )

    # tile -> slot permutation per core (align heavy tiles across cores)
    perm = np.argsort(-counts0.sum(axis=2), axis=1, kind="stable")  # [cores, tiles]
    counts = np.take_along_axis(counts0, perm[:, :, None], axis=1)  # [cores,slot,b]

    # ---- runs: (supertile, block) -> concatenated slot buckets, pad to 128
    chunk_block = []
    chunk_super = []
    run_meta = []  # (s, b, chunk0, nch, ends_k [cores, nts], ts)
    for s in range(n_super):
        ts = list(range(s * stile, min((s + 1) * stile, tiles)))
        for b in range(nblocks):
            c_kt = counts[:, ts, b]                      # [cores, nts]
            run_max = int(c_kt.sum(axis=1).max())
            if run_max == 0:
                continue
            nch = -(-run_max // P)
            chunk0 = len(chunk_block)
            chunk_block += [b] * nch
            chunk_super += [s] * nch
            run_meta.append((s, b, chunk0, nch, np.cumsum(c_kt, axis=1), ts))
    ch = len(chunk_block)
    chunk_block = np.array(chunk_block)
    chunk_super = np.array(chunk_super)

    # ---- matmul slots: union over cores of tiles present in each chunk
    mm_chunk = []
    mm_tile = []
    for s, b, chunk0, nch, ends_k, ts in run_meta:
        starts_k = ends_k - counts[:, ts, b]
        for ci_local in range(nch):
            a0, a1 = ci_local * P, (ci_local + 1) * P
            present = ((starts_k < a1) & (ends_k > a0)).any(axis=0)
            for j in np.nonzero(present)[0]:
                mm_chunk.append(chunk0 + ci_local)
                mm_tile.append(ts[j])
    nslots = len(mm_chunk)
    mm_chunk = np.array(mm_chunk)
    mm_tile = np.array(mm_tile)

    mm_first = np.zeros(nslots, dtype=bool)
    mm_last = np.zeros(nslots, dtype=bool)
    seen = set()
    for i in range(nslots):
        t = int(mm_tile[i])
        if t not in seen:
            seen.add(t)
            mm_first[i] = True
    seen = set()
    for i in range(nslots - 1, -1, -1):
        t = int(mm_tile[i])
        if t not in seen:
            seen.add(t)
            mm_last[i] = True
    tile_has = np.zeros(tiles, dtype=bool)
    if nslots:
        tile_has[np.unique(mm_tile)] = True

    # ---- calls: one call per run; two padded sizes (w_lo/w_hi) minimize
    # zero-pad emission while keeping num_idxs a shared constant per size
    sizes = sorted({nch for _, _, _, nch, _, _ in run_meta})
    w_hi = max(sizes)
    best = (None, None)
    for wl in sizes:
        pad = sum(
            (wl - nch) if nch <= wl else (w_hi - nch)
            for _, _, _, nch, _, _ in run_meta
        )
        if best[0] is None or pad < best[0]:
            best = (pad, wl)
    w_lo = best[1]
    calls = []  # (block, c0, csize, slot0, nslots_call, wcall, off)
    off = 0
    for s, b, chunk0, nch, ends_k, ts in run_meta:
        s0 = int(np.searchsorted(mm_chunk, chunk0))
        s1 = int(np.searchsorted(mm_chunk, chunk0 + nch))
        wcall = w_lo if nch <= w_lo else w_hi
        calls.append((b, chunk0, nch, s0, s1 - s0, wcall, off))
        off += wcall
    total_w = off
    max_slots_call = max(c[4] for c in calls)

    # ---- per-core streams
    idx_flat = np.zeros((n_cores, ch * P), np.int16)
    ldst_slots = np.full((n_cores, nslots, P), -1.0, BF)
    for k in range(n_cores):
        for s, b, chunk0, nch, ends_k, ts in run_meta:
            pieces_src = []
            pieces_ldst = []
            pieces_slot = []
            for j, t in enumerate(ts):
                cnt = int(counts[k, t, b])
                if cnt == 0:
                    continue
                tt = int(perm[k, t])
                g0 = int(all_starts[(k * tiles + tt) * nblocks + b])
                pieces_src.append(src_s[g0 : g0 + cnt] - b * block)
                pieces_ldst.append(dst_s[g0 : g0 + cnt] - (k * npc + tt * P))
                pieces_slot.append(np.full(cnt, t, np.int64))
            if not pieces_src:
                continue
            esrc = np.concatenate(pieces_src).astype(np.int16)
            eldst = np.concatenate(pieces_ldst)
            eslot = np.concatenate(pieces_slot)
            n_e = esrc.shape[0]
            base = chunk0 * P
            idx_flat[k, base : base + n_e] = esrc
            s0 = int(np.searchsorted(mm_chunk, chunk0))
            s1 = int(np.searchsorted(mm_chunk, chunk0 + nch))
            for i in range(s0, s1):
                ci_local = int(mm_chunk[i]) - chunk0
                t = int(mm_tile[i])
                a0 = ci_local * P
                a1 = min(a0 + P, n_e)
                if a0 >= n_e:
                    continue
                m = eslot[a0:a1] == t
                if not m.any():
                    continue
                col = ldst_slots[k, i]
                col[: a1 - a0][m] = eldst[a0:a1][m].astype(BF)

    # wrap indices into 16 partitions, replicate across the 8 core groups
    idx_wrapped = np.tile(
        idx_flat.reshape(n_cores, ch * 8, 16).transpose(0, 2, 1), (1, 8, 1)
    )  # [cores, 128, ch*8]

    # per-call full-width idx stream: call j occupies cols [j*w*8,(j+1)*w*8);
    # tail cols beyond csize*8 are 0 (a valid row — gathered then discarded;
    # trailing -1 trim corrupts ring bookkeeping when queues are reused), so
    # every call passes the same num_idxs = w*128 and shares one register.
    ncalls = len(calls)
    idx_calls = np.zeros((n_cores, P, total_w * 8), np.int16)
    for b, c0, csize, s0, nsc, wcall, off in calls:
        idx_calls[:, :, off * 8 : off * 8 + csize * 8] = idx_wrapped[
            :, :, c0 * 8 : (c0 + csize) * 8
        ]
    idx_all = np.ascontiguousarray(idx_calls)

    # ldst packed at a fixed per-call stride (msc slots) so meta DMAs can
    # fetch G calls' worth in one transfer
    ldst_t_slots = ldst_slots.transpose(0, 2, 1)  # [cores, P, nslots]
    ldst_pack = np.full((n_cores, P, ncalls * max_slots_call), -1.0, BF)
    for j, (b, c0, csize, s0, nsc, wcall, off) in enumerate(calls):
        if nsc:
            ldst_pack[:, :, j * max_slots_call : j * max_slots_call + nsc] = (
                ldst_t_slots[:, :, s0 : s0 + nsc]
            )
            # slots offloaded to ScalarE (last ~20% per call) store -l so the
            # activation bias computes (iota - l) directly
            v_cnt = nsc - nsc * SCALAR_PCT // 100
            if v_cnt < nsc:
                cols = slice(j * max_slots_call + v_cnt, j * max_slots_call + nsc)
                ldst_pack[:, :, cols] = -ldst_pack[:, :, cols]
    ldst_all = np.ascontiguousarray(ldst_pack)

    return dict(
        npc=npc,
        tiles=tiles,
        nblocks=nblocks,
        n_super=n_super,
        stile=stile,
        ch=ch,
        nslots=nslots,
        calls=calls,
        max_slots_call=max_slots_call,
        chunk_super=chunk_super,
        mm_chunk=mm_chunk,
        mm_tile=mm_tile,
        mm_first=mm_first,
        mm_last=mm_last,
        tile_has_chunks=tile_has,
        idx=idx_all,
        w_lo=w_lo,
        w_hi=w_hi,
        total_w=total_w,
        ldst=ldst_all,
        perm=perm,
    )


def _pack_x(x):
    """[N, D] f32 -> [N, 2D] bf16 rows: [hi | lo]."""
    x = np.asarray(x, np.float32)
    hi = x.astype(BF)
    lo = (x - hi.astype(np.float32)).astype(BF)
    return np.ascontiguousarray(np.concatenate([hi, lo], axis=1))


def _build(n_nodes, dim, block, w, sched):
    w_lo = sched["w_lo"]
    w_hi = sched["w_hi"]
    total_w = sched["total_w"]
    tiles = sched["tiles"]
    stile = sched["stile"]
    n_super = sched["n_super"]
    nslots = sched["nslots"]
    calls = sched["calls"]
    msc = sched["max_slots_call"]
    chunk_super = sched["chunk_super"]
    mm_chunk = sched["mm_chunk"]
    mm_tile = sched["mm_tile"]
    mm_first = sched["mm_first"]
    mm_last = sched["mm_last"]
    tile_has = sched["tile_has_chunks"]
    ncalls = len(calls)
    out_pad = tiles * P
    elem = 2 * dim  # packed bf16 row length

    nc = bacc.Bacc(
        "TRN2", target_bir_lowering=False, debug=False, num_swdge_queues=N_QUEUES
    )
    x_t = nc.dram_tensor("xpack", [n_nodes, elem], BF16, kind="ExternalInput")
    idx_t = nc.dram_tensor("idx", [P, total_w * 8], I16, kind="ExternalInput")
    ldst_t = nc.dram_tensor("ldst", [P, ncalls * msc], BF16, kind="ExternalInput")
    out_t = nc.dram_tensor("out", [out_pad, dim], F32, kind="ExternalOutput")
    G = 4  # calls per meta DMA group

    with tile.TileContext(nc) as tc:
        with (
            tc.tile_pool(name="const", bufs=1) as const_pool,
            tc.tile_pool(name="meta", bufs=8) as meta_pool,
            tc.tile_pool(name="gather", bufs=8) as gather_pool,
            tc.tile_pool(name="oh", bufs=14) as oh_pool,
            tc.tile_pool(name="sq", bufs=16) as sq_pool,
            tc.tile_pool(name="stage", bufs=1) as stage_pool,
            tc.tile_pool(name="psum", bufs=8, space="PSUM") as psum_pool,
        ):
            iota_i = const_pool.tile([P, 8 * P], I32)
            nc.gpsimd.iota(
                iota_i[:], pattern=[[0, 8], [1, P]], base=0, channel_multiplier=0
            )
            iota_b = const_pool.tile([P, 8 * P], BF16)
            nc.vector.tensor_copy(iota_b[:], iota_i[:])

            stage = stage_pool.tile([P, tiles * dim], F32)
            if not tile_has.all():
                nc.vector.memset(stage[:], 0.0)
            out_view = out_t[:, :].rearrange("(t p) d -> p t d", p=P)

            # shared num_idxs registers: written once, read by every
            # gather -> no per-call MOVE / WAR hazard serializing dispatch
            reg_lo = nc.gpsimd.to_reg(w_lo * P)
            reg_hi = nc.gpsimd.to_reg(w_hi * P)

            call_idx = 0
            psums = {}
            for s in range(n_super):
                ts = list(range(s * stile, min((s + 1) * stile, tiles)))
                while call_idx < len(calls):
                    b, c0, csize, s0, nsc, wcall, off = calls[call_idx]
                    if int(chunk_super[c0]) != s:
                        break
                    queue = QUEUE_PATTERN[call_idx % len(QUEUE_PATTERN)]
                    j = call_idx
                    call_idx += 1
                    if j % G == 0:
                        ng = min(G, ncalls - j)
                        goff = off
                        gw = sum(c[5] for c in calls[j : j + ng])
                        idx_gtile = meta_pool.tile(
                            [P, G * w_hi * 8], I16, tag="idx"
                        )
                        nc.sync.dma_start(
                            idx_gtile[:, : gw * 8],
                            idx_t[:, goff * 8 : (goff + gw) * 8],
                        )
                        ldst_gtile = meta_pool.tile([P, G * msc], BF16, tag="ldst")
                        nc.sync.dma_start(
                            ldst_gtile[:, : ng * msc],
                            ldst_t[:, j * msc : (j + ng) * msc],
                        )
                    jg = j % G
                    coff = off - goff
                    msgs = gather_pool.tile([P, w_hi, elem], BF16)
                    nc.gpsimd.dma_gather(
                        out_ap=msgs[:, :wcall, :],
                        in_ap=x_t[b * block : min((b + 1) * block, n_nodes), :],
                        idxs_ap=idx_gtile[:, coff * 8 : (coff + wcall) * 8],
                        num_idxs=wcall * P,
                        num_idxs_reg=reg_lo if wcall == w_lo else reg_hi,
                        elem_size=elem,
                        single_packet=SINGLE_PACKET,
                        queue_num=queue,
                    )
                    v_cnt = nsc - nsc * SCALAR_PCT // 100

                    def do_mm(si, lhsT_ap, cin):
                        t = int(mm_tile[si])
                        if mm_first[si]:
                            psums[t] = psum_pool.tile(
                                [P, elem], F32, tag="ps", name=f"ps{t}"
                            )
                        nc.tensor.matmul(
                            psums[t][:, :],
                            lhsT=lhsT_ap,
                            rhs=msgs[:, cin, :],
                            start=bool(mm_first[si]),
                            stop=bool(mm_last[si]),
                        )

                    for j0 in range(0, v_cnt, 8):
                        g = min(8, v_cnt - j0)
                        onehot = oh_pool.tile([P, 8 * P], BF16, name="oh", tag="oh")
                        lt = ldst_gtile[:, jg * msc + j0 : jg * msc + j0 + g]
                        lt_b = bass.AP(lt.tensor, lt.offset, lt.ap + [[0, P]])
                        nc.vector.tensor_tensor(
                            out=onehot[:, : g * P].rearrange(
                                "p (g q) -> p g q", q=P
                            ),
                            in0=iota_b[:, : g * P].rearrange(
                                "p (g q) -> p g q", q=P
                            ),
                            in1=lt_b,
                            op=mybir.AluOpType.is_equal,
                        )
                        for jj in range(g):
                            si = s0 + j0 + jj
                            cin = int(mm_chunk[si]) - c0
                            do_mm(si, onehot[:, jj * P : (jj + 1) * P], cin)
                    for j0 in range(v_cnt, nsc):
                        si = s0 + j0
                        cin = int(mm_chunk[si]) - c0
                        lt_col = ldst_gtile[:, jg * msc + j0 : jg * msc + j0 + 1]
                        sq = sq_pool.tile([P, P], BF16, tag="sq")
                        nc.scalar.activation(
                            sq[:],
                            iota_b[:, :P],
                            mybir.ActivationFunctionType.Square,
                            bias=lt_col,
                        )
                        oh_s = sq_pool.tile([P, P], BF16, tag="ohs")
                        nc.scalar.activation(
                            oh_s[:],
                            sq[:],
                            mybir.ActivationFunctionType.Relu,
                            bias=1.0,
                            scale=-1.0,
                        )
                        do_mm(si, oh_s[:], cin)
                # evacuate: stage[:, t*dim:+dim] = psum_hi + psum_lo
                for t in ts:
                    if not tile_has[t]:
                        continue
                    ps = psums.pop(t)
                    nc.scalar.copy(stage[:, t * dim : (t + 1) * dim], ps[:, :dim])
                    nc.vector.tensor_tensor(
                        out=stage[:, t * dim : (t + 1) * dim],
                        in0=stage[:, t * dim : (t + 1) * dim],
                        in1=ps[:, dim:],
                        op=mybir.AluOpType.add,
                    )
                # stream this supertile's rows out while later ones compute
                t0, t1 = ts[0], ts[-1] + 1
                nc.sync.dma_start(
                    out_view[:, t0:t1, :],
                    stage[:, t0 * dim : t1 * dim].rearrange(
                        "p (t d) -> p t d", d=dim
                    ),
                )

    nc.compile()
    return nc


def _run(x, edge_index, n_nodes, dim, n_cores, block, w, **run_kwargs):
    sched = _prep(edge_index, n_nodes, n_cores, block, w)
    xp = _pack_x(x)
    nc = _build(n_nodes, dim, block, sched["w_hi"], sched)
    in_maps = [
        {"xpack": xp, "idx": sched["idx"][k], "ldst": sched["ldst"][k]}
        for k in range(n_cores)
    ]
    res = run_bass_kernel_spmd(
        nc, in_maps, core_ids=list(range(n_cores)), **run_kwargs
    )
    npc = sched["npc"]
    tiles = sched["tiles"]
    perm = sched["perm"]
    parts = []
    for k in range(n_cores):
        r = res.results[k]["out"].reshape(tiles, P, -1)
        inv = np.empty(tiles, np.int64)
        inv[perm[k]] = np.arange(tiles)
        parts.append(r[inv].reshape(tiles * P, -1)[:npc])
    out = np.concatenate(parts, axis=0)
    return out, res


def kernel(x, edge_index):
    out, _ = _run(
        x, edge_index, N_NODES, DIM, N_CORES, SRC_BLOCK, CHUNKS_PER_CALL
    )
    return out
